# revision 6
# baseline (speedup 1.0000x reference)
"""Haar 2D DWT (pywt 'haar') Trainium2 Bass kernel.

Full input x: [16, 64, 256, 256] f32.
Output: [16, 256, 128, 128] f32 = concat(ll, lh, hl, hh) on channel axis.

Sharding: pure data-parallel over batch (16 -> 2 per core x 8 cores).

Per-core layout: partition p = image index (b*C + c) -- exactly 128
images per core. Free dim = a band of R row-pairs of that image
(R*512 contiguous f32 per partition per load). This makes every DMA
descriptor a large contiguous run (R*2KB in, R*512B out per partition).

Per band:
  - stage 0: scale whole tile by 0.5 (DVE tensor_scalar, 2x_2P f32 mode)
  - stage 1 (row butterfly): s = even_row + odd_row, d = even_row - odd_row
  - stage 2 (col butterfly): ll = s_e + s_o, lh = d_e + d_o,
                             hl = s_e - s_o, hh = d_e - d_o
"""

import numpy as np

N_CORES = 8
FULL_B, C, H, W = 16, 64, 256, 256


def _build_bass(B=2, Cc=64, Hh=256, Ww=256, R=8, bufs=3):
    import concourse.bacc as bacc
    import concourse.mybir as mybir
    from concourse.tile import TileContext

    P = B * Cc           # partitions = images per core
    HP = Hh // 2         # row pairs per image
    Wh = Ww // 2
    f32 = mybir.dt.float32
    assert HP % R == 0

    nc = bacc.Bacc("TRN2", target_bir_lowering=False, debug=False)
    x = nc.dram_tensor("x", [B, Cc, Hh, Ww], f32, kind="ExternalInput").ap()
    y = nc.dram_tensor("y", [B, 4 * Cc, HP, Wh], f32, kind="ExternalOutput").ap()

    # [P, H*W]: one whole image per partition row
    xi = x.rearrange("b c h w -> (b c) (h w)")
    # [4, B, C, HP*Wh]: quadrant-major view of the output images
    yo = y.rearrange("b (q c) h w -> q b c (h w)", q=4)

    with TileContext(nc) as tc:
        with tc.tile_pool(name="pool", bufs=bufs) as pool:
            for r0 in range(0, HP, R):
                in_t = pool.tile([P, R * Ww * 2], f32, tag="in")
                nc.sync.dma_start(
                    out=in_t[:], in_=xi[:, r0 * 2 * Ww : (r0 + R) * 2 * Ww]
                )
                nc.vector.tensor_scalar_mul(in_t[:], in_t[:], 0.5)
                iv = in_t[:].rearrange("p (r t w) -> p r t w", r=R, t=2)
                s_t = pool.tile([P, R * Ww], f32, tag="s")
                d_t = pool.tile([P, R * Ww], f32, tag="d")
                sv = s_t[:].rearrange("p (r w) -> p r w", r=R)
                dv = d_t[:].rearrange("p (r w) -> p r w", r=R)
                nc.vector.tensor_add(out=sv, in0=iv[:, :, 0, :], in1=iv[:, :, 1, :])
                nc.vector.tensor_sub(out=dv, in0=iv[:, :, 0, :], in1=iv[:, :, 1, :])
                sp = s_t[:].rearrange("p (r w t) -> p r w t", r=R, t=2)
                dp = d_t[:].rearrange("p (r w t) -> p r w t", r=R, t=2)
                o_ts = [
                    pool.tile([P, R * Wh], f32, name=f"o{q}", tag=f"o{q}")
                    for q in range(4)
                ]
                ovs = [o[:].rearrange("p (r w) -> p r w", r=R) for o in o_ts]
                nc.vector.tensor_add(out=ovs[0], in0=sp[:, :, :, 0], in1=sp[:, :, :, 1])
                nc.vector.tensor_add(out=ovs[1], in0=dp[:, :, :, 0], in1=dp[:, :, :, 1])
                nc.vector.tensor_sub(out=ovs[2], in0=sp[:, :, :, 0], in1=sp[:, :, :, 1])
                nc.vector.tensor_sub(out=ovs[3], in0=dp[:, :, :, 0], in1=dp[:, :, :, 1])
                for q in range(4):
                    nc.sync.dma_start(
                        out=yo[q][:, :, r0 * Wh : (r0 + R) * Wh], in_=o_ts[q][:]
                    )
    nc.compile()
    return nc


def kernel(x: np.ndarray) -> np.ndarray:
    from concourse.bass_utils import run_bass_kernel_spmd

    x = np.ascontiguousarray(np.asarray(x, dtype=np.float32))
    assert x.shape == (FULL_B, C, H, W), x.shape
    nc = _build_bass()
    shards = np.split(x, N_CORES, axis=0)
    in_maps = [{"x": s} for s in shards]
    res = run_bass_kernel_spmd(nc, in_maps, list(range(N_CORES)))
    return np.concatenate([r["y"] for r in res.results], axis=0)


# revision 9
# speedup vs baseline: 3.2076x; 3.2076x over previous
"""Haar 2D DWT (pywt 'haar') Trainium2 Bass kernel.

Full input x: [16, 64, 256, 256] f32.
Output: [16, 256, 128, 128] f32 = concat(ll, lh, hl, hh) on channel axis.

Sharding: pure data-parallel over batch (16 -> 2 per core x 8 cores).

Per-core layout: partition p = image index (b*C + c) -- exactly 128
images per core. Free dim = a band of R row-pairs of that image
(R*512 contiguous f32 per partition per load). This makes every DMA
descriptor a large contiguous run (R*2KB in, R*512B out per partition).

Per band:
  - stage 0: scale whole tile by 0.5 (DVE tensor_scalar, 2x_2P f32 mode)
  - stage 1 (row butterfly): s = even_row + odd_row, d = even_row - odd_row
  - stage 2 (col butterfly): ll = s_e + s_o, lh = d_e + d_o,
                             hl = s_e - s_o, hh = d_e - d_o
"""

import numpy as np

N_CORES = 8
FULL_B, C, H, W = 16, 64, 256, 256


def _build_bass(B=2, Cc=64, Hh=256, Ww=256, R=8, bufs=3):
    import concourse.bacc as bacc
    import concourse.mybir as mybir
    from concourse.tile import TileContext

    P = B * Cc           # partitions = images per core
    HP = Hh // 2         # row pairs per image
    Wh = Ww // 2
    f32 = mybir.dt.float32
    assert HP % R == 0

    nc = bacc.Bacc("TRN2", target_bir_lowering=False, debug=False)
    x = nc.dram_tensor("x", [B, Cc, Hh, Ww], f32, kind="ExternalInput").ap()
    y = nc.dram_tensor("y", [B, 4 * Cc, HP, Wh], f32, kind="ExternalOutput").ap()

    # [C, B, H*W]: one whole image per partition row, c-major partition
    # order so every DMA's outermost AP dim has count C (HWDGE fans a DMA
    # out across SDMA engines by the outer dim -- count B=2 would use 2
    # of 16 engines).
    xi = x.rearrange("b c h w -> c b (h w)")
    # [4, C, B, HP*Wh]: quadrant-major view of the output images
    yo = y.rearrange("b (q c) h w -> q c b (h w)", q=4)

    with TileContext(nc) as tc:
        with tc.tile_pool(name="pool", bufs=bufs) as pool:
            for r0 in range(0, HP, R):
                in_t = pool.tile([P, R * Ww * 2], f32, tag="in")
                nc.sync.dma_start(
                    out=in_t[:], in_=xi[:, :, r0 * 2 * Ww : (r0 + R) * 2 * Ww]
                )
                nc.scalar.mul(in_t[:], in_t[:], 0.5)
                iv = in_t[:].rearrange("p (r t w) -> p r t w", r=R, t=2)
                s_t = pool.tile([P, R * Ww], f32, tag="s")
                d_t = pool.tile([P, R * Ww], f32, tag="d")
                sv = s_t[:].rearrange("p (r w) -> p r w", r=R)
                dv = d_t[:].rearrange("p (r w) -> p r w", r=R)
                nc.vector.tensor_add(out=sv, in0=iv[:, :, 0, :], in1=iv[:, :, 1, :])
                nc.vector.tensor_sub(out=dv, in0=iv[:, :, 0, :], in1=iv[:, :, 1, :])
                sp = s_t[:].rearrange("p (r w t) -> p r w t", r=R, t=2)
                dp = d_t[:].rearrange("p (r w t) -> p r w t", r=R, t=2)
                o_ts = [
                    pool.tile([P, R * Wh], f32, name=f"o{q}", tag=f"o{q}")
                    for q in range(4)
                ]
                ovs = [o[:].rearrange("p (r w) -> p r w", r=R) for o in o_ts]
                nc.vector.tensor_add(out=ovs[0], in0=sp[:, :, :, 0], in1=sp[:, :, :, 1])
                nc.vector.tensor_add(out=ovs[1], in0=dp[:, :, :, 0], in1=dp[:, :, :, 1])
                nc.vector.tensor_sub(out=ovs[2], in0=sp[:, :, :, 0], in1=sp[:, :, :, 1])
                nc.vector.tensor_sub(out=ovs[3], in0=dp[:, :, :, 0], in1=dp[:, :, :, 1])
                for q in range(4):
                    nc.sync.dma_start(
                        out=yo[q][:, :, r0 * Wh : (r0 + R) * Wh], in_=o_ts[q][:]
                    )
    nc.compile()
    return nc


def kernel(x: np.ndarray) -> np.ndarray:
    from concourse.bass_utils import run_bass_kernel_spmd

    x = np.ascontiguousarray(np.asarray(x, dtype=np.float32))
    assert x.shape == (FULL_B, C, H, W), x.shape
    nc = _build_bass()
    shards = np.split(x, N_CORES, axis=0)
    in_maps = [{"x": s} for s in shards]
    res = run_bass_kernel_spmd(nc, in_maps, list(range(N_CORES)))
    return np.concatenate([r["y"] for r in res.results], axis=0)
